# revision 1
# baseline (speedup 1.0000x reference)
"""BinarizeLinear Trainium2 kernel.

Computes out = x @ sign(W).T + bias for x [262144, 512], W [512, 512],
bias [512], data-parallel over 8 NeuronCores (x sharded along rows).

Strategy per core (shard = 32768 rows):
  - PE runs fp8e4m3 matmuls in DoubleRow perf mode (2 MACs/cell/cycle).
    The DoubleRow pack dimension carries a hi/lo split of x:
    slot 0 = e4m3(x) against w, slot 1 = e4m3(16*(x - hi)) against w/16,
    so one DoubleRow matmul accumulates hi*w + lo*w/16 ~= x*w at ~bf16+
    precision into fp32 PSUM, at the bf16 cycle count. sign(W) is +-1 and
    w/16 is +-2^-4 - both exact in e4m3.
  - Host prep: x shard pre-tiled+packed into per-block, per-ko contiguous
    chunks [ko][ki=128, j=2, ns, p] fp8 so every DMA read segment is one
    contiguous run per partition AND the first matmul group of a block
    only waits on a quarter of the block's bytes. Output is written bf16
    and upcast to fp32 on host.
  - Device: per block, one x DMA per ko (sync/SP HWDGE ring), 4
    accumulating DoubleRow matmuls per 128-row subtile (lhsT = x pack
    [128,2,128], rhs = w pack [128,2,512], PSUM [128 n, 512 o]),
    bias-add on DVE copying PSUM -> SBUF bf16, one out-DMA per block on
    the scalar/ACT HWDGE ring (separate ring from reads).
  - n-assignment interleaved (lhsT column p of subtile s covers row
    p*n_sub + s) so each partition's output rows are consecutive ->
    one contiguous DRAM write segment per partition per block.
  - Block sizes ramp at start/end to shorten pipeline fill/drain; ~40
    dependency-free warmup matmuls run during the DMA fill to start the
    PE HAM clock-gate ramp early.
"""

import numpy as np
import ml_dtypes

import concourse.mybir as mybir
from concourse import bacc, bass_utils
from concourse.tile import TileContext

N_CORES = 8
N_TOTAL = 262144
IN_F = 512
OUT_F = 512
N_SHARD = N_TOTAL // N_CORES  # 32768
K_BLOCKS = IN_F // 128        # 4
P = 128
J = 2                         # DoubleRow pack: hi/lo

# ramped block schedule (rows per block); sums to N_SHARD
BLOCKS = [256, 256, 512] + [1024] * 30 + [512, 256, 256]
assert sum(BLOCKS) == N_SHARD

SPLIT_KO = True  # one x-DMA per ko block (finer matmul-ready granularity)

_nc_cache = None


def _build_nc():
    nc = bacc.Bacc(
        "TRN2", target_bir_lowering=False, debug=False, num_devices=N_CORES
    )
    # x pre-packed on host: per block, per ko a contiguous [128, 2*blk] chunk
    xt_d = nc.dram_tensor(
        "xt", [N_SHARD * IN_F * J], mybir.dt.float8e4, kind="ExternalInput"
    ).ap()
    wt_d = nc.dram_tensor(
        "wt", [P, K_BLOCKS, J, OUT_F], mybir.dt.float8e4, kind="ExternalInput"
    ).ap()
    b_d = nc.dram_tensor(
        "bias_bcast", [P, OUT_F], mybir.dt.bfloat16, kind="ExternalInput"
    ).ap()
    out_d = nc.dram_tensor(
        "out", [N_SHARD, OUT_F], mybir.dt.bfloat16, kind="ExternalOutput"
    ).ap()

    with TileContext(nc) as tc:
        with (
            tc.tile_pool(name="const", bufs=1) as cpool,
            tc.tile_pool(name="xin", bufs=4) as xpool,
            tc.tile_pool(name="outp", bufs=4) as opool,
            tc.tile_pool(name="psum", bufs=7, space="PSUM") as ppool,
            tc.tile_pool(name="warm", bufs=1, space="PSUM") as wpool,
        ):
            # dependency-free dummy matmuls on a zeroed SBUF tile: they
            # schedule at engine boot and hold the PE busy so the HAM
            # clock-gate ramp starts before the first real matmul
            scratch = cpool.tile([P, P], mybir.dt.bfloat16)
            nc.gpsimd.memset(scratch[:], 0.0)
            wps = wpool.tile([P, 64], mybir.dt.float32)
            for _ in range(40):
                nc.tensor.matmul(
                    wps[:], lhsT=scratch[:], rhs=scratch[:, :64],
                    start=True, stop=True,
                )

            # constants on the ACT (write) ring so the first x-block
            # read isn't queued behind them on the SP ring
            wt_sb = cpool.tile([P, K_BLOCKS, J, OUT_F], mybir.dt.float8e4)
            nc.scalar.dma_start(wt_sb[:], wt_d[:])
            b_sb = cpool.tile([P, OUT_F], mybir.dt.bfloat16)
            nc.scalar.dma_start(b_sb[:], b_d[:])

            off = 0
            for bi, blk in enumerate(BLOCKS):
                n_sub = blk // P
                x_sb = [
                    xpool.tile([P, J, n_sub, P], mybir.dt.float8e4,
                               tag=f"x{ko}", name=f"x{ko}")
                    for ko in range(K_BLOCKS)
                ]
                base = off * IN_F * J
                ko_sz = blk * P * J  # elements per ko chunk
                for ko in range(K_BLOCKS):
                    src = xt_d[
                        base + ko * ko_sz:base + (ko + 1) * ko_sz
                    ].rearrange("(ki f) -> ki f", ki=P)
                    nc.sync.dma_start(
                        x_sb[ko][:].rearrange("p j s q -> p (j s q)"), src
                    )
                o_sb = opool.tile([P, n_sub, OUT_F], mybir.dt.bfloat16)
                # rows [off, off+blk) as [p, s, o]: row = off + p*n_sub + s
                # -> contiguous (s, o) run per partition
                dst = out_d[off:off + blk, :].rearrange(
                    "(p s) o -> p s o", s=n_sub
                )
                # write each block in halves so the first half's out-DMA
                # overlaps the second half's matmuls
                h = max(1, min(4, n_sub // 2))
                for half in range((n_sub + h - 1) // h):
                    s0, s1 = half * h, min((half + 1) * h, n_sub)
                    for ns in range(s0, s1):
                        ps = ppool.tile([P, OUT_F], mybir.dt.float32)
                        for ko in range(K_BLOCKS):
                            # column p covers row off + p*n_sub + ns
                            nc.tensor.matmul(
                                ps[:],
                                lhsT=x_sb[ko][:, :, ns, :],
                                rhs=wt_sb[:, ko, :, :],
                                start=(ko == 0),
                                stop=(ko == K_BLOCKS - 1),
                                perf_mode=mybir.MatmulPerfMode.DoubleRow,
                            )
                        nc.vector.tensor_add(o_sb[:, ns, :], ps[:], b_sb[:])
                    nc.scalar.dma_start(
                        dst[:, s0:s1, :], o_sb[:, s0:s1, :]
                    )
                off += blk

    nc.finalize()
    return nc


_E4 = ml_dtypes.float8_e4m3


def _pack_x_shard(shard_f32: np.ndarray) -> np.ndarray:
    """[N_SHARD, 512] fp32 -> flat fp8 per-block [ko][ki, j, ns, p] pack."""
    chunks = []
    off = 0
    for blk in BLOCKS:
        n_sub = blk // P
        b = shard_f32[off:off + blk, :].reshape(P, n_sub, K_BLOCKS, P)
        # axes: [p, ns, ko, ki]
        hi = b.astype(_E4)
        lo = ((b - hi.astype(np.float32)) * 16.0).astype(_E4)
        pack = np.stack([hi, lo], axis=0)        # [j, p, ns, ko, ki]
        pack = pack.transpose(3, 4, 0, 2, 1)     # [ko, ki, j, ns, p]
        chunks.append(np.ascontiguousarray(pack).reshape(-1))
        off += blk
    return np.concatenate(chunks)


def kernel(x: np.ndarray, weight: np.ndarray, bias: np.ndarray, **run_kwargs):
    global _nc_cache
    if _nc_cache is None:
        _nc_cache = _build_nc()
    nc = _nc_cache

    x = np.asarray(x)
    weight = np.asarray(weight)
    bias = np.asarray(bias)

    wb = np.sign(weight.astype(np.float32)).T          # [512 i, 512 o]
    wbr = wb.reshape(K_BLOCKS, P, OUT_F)               # [ko, ki, o]
    wt = np.stack(
        [wbr.astype(_E4), (wbr / 16.0).astype(_E4)], axis=2
    )                                                  # [ko, ki, j, o]
    wt = np.ascontiguousarray(wt.transpose(1, 0, 2, 3))  # [ki, ko, j, o]
    bias_bcast = np.ascontiguousarray(
        np.broadcast_to(bias.astype(ml_dtypes.bfloat16)[None, :], (P, OUT_F))
    )

    in_maps = []
    for c in range(N_CORES):
        shard = np.ascontiguousarray(
            x[c * N_SHARD:(c + 1) * N_SHARD, :], dtype=np.float32
        )
        in_maps.append(
            {"xt": _pack_x_shard(shard), "wt": wt, "bias_bcast": bias_bcast}
        )

    res = bass_utils.run_bass_kernel_spmd(
        nc, in_maps, core_ids=list(range(N_CORES)), **run_kwargs
    )
    out = np.empty((N_TOTAL, OUT_F), dtype=np.float32)
    for c in range(N_CORES):
        out[c * N_SHARD:(c + 1) * N_SHARD, :] = res.results[c]["out"].astype(
            np.float32
        )
    if run_kwargs:
        kernel.last_result = res
    return out



# revision 3
# speedup vs baseline: 1.3057x; 1.3057x over previous
"""BinarizeLinear Trainium2 kernel.

Computes out = x @ sign(W).T + bias for x [262144, 512], W [512, 512],
bias [512], data-parallel over 8 NeuronCores (x sharded along rows).

Strategy per core (shard = 32768 rows):
  - PE runs fp8e4m3 matmuls in DoubleRow perf mode (2 MACs/cell/cycle).
    Unlike the 4-matmul hi/lo scheme, the DoubleRow pack dimension here
    carries REAL contraction: per 128-row subtile only 3 DoubleRow
    matmuls (2 main + 1 dither) cover the K=512 contraction:
      MM t=0/1: lhsT = e4m3 main codes a_f for features 256t+2ki+j,
                rhs = sign(W) (+-1, exact in e4m3).
      MM t=2:   a shared "dither" slot d_m per feature pair (2m, 2m+1),
                rhs slot m = h*(w_{2m} + 0.5*w_{2m+1}) with h = 2^-6
                (values +-1.5h/+-0.5h, exact in e4m3). Slot m=255 is a
                constant-1 column against rhs = e4m3(bias): the bias add
                rides the matmul for free.
    Effective x~_2m = a_2m + h*d_m, x~_2m+1 = a_2m+1 + (h/2)*d_m. The
    host encoder jointly picks (a_p, a_q, d_m) per pair (exact e4m3
    search over ~11 sweet-spot dither candidates), cutting quantization
    error ~4.5x below plain e4m3 (rel err ~0.6% << 2% gate) while
    keeping 1.5 fp8 bytes/feature and 3/4 of the baseline's matmuls.
  - Host prep: x shard pre-tiled+packed into per-block, per-t contiguous
    chunks [t][ki=128, j=2, ns, p] fp8 so every DMA read segment is one
    contiguous run per partition. Output is written bf16 and upcast to
    fp32 on host.
  - Device: per block, one x DMA per t (sync/SP HWDGE ring), 3
    accumulating DoubleRow matmuls per 128-row subtile (lhsT = x pack
    [128,2,128], rhs = w pack [128,2,512], PSUM [128 n, 512 o]), DVE
    copy PSUM -> SBUF bf16 (bias already added in PE), one out-DMA per
    half-block on the scalar/ACT HWDGE ring (separate ring from reads).
  - n-assignment interleaved (lhsT column p of subtile s covers row
    p*n_sub + s) so each partition's output rows are consecutive ->
    one contiguous DRAM write segment per partition per block.
  - Block sizes ramp at start/end to shorten pipeline fill/drain; ~40
    dependency-free warmup matmuls run during the DMA fill to start the
    PE HAM clock-gate ramp early.
"""

import numpy as np
import ml_dtypes

import concourse.mybir as mybir
from concourse import bacc, bass_utils
from concourse.tile import TileContext

N_CORES = 8
N_TOTAL = 262144
IN_F = 512
OUT_F = 512
N_SHARD = N_TOTAL // N_CORES  # 32768
P = 128
J = 2                         # DoubleRow pack dim
T_MM = 3                      # matmuls per subtile: 2 main + 1 dither
N_PAIR = 256                  # feature pairs per row

H = np.float32(2.0 ** -6)     # dither scale for pair member p
KH = np.float32(2.0 ** -7)    # dither scale for pair member q

# ramped block schedule (rows per block); sums to N_SHARD
BLOCKS = [256, 256, 512] + [1024] * 30 + [512, 256, 256]
assert sum(BLOCKS) == N_SHARD

_nc_cache = None


def _build_nc():
    nc = bacc.Bacc(
        "TRN2", target_bir_lowering=False, debug=False, num_devices=N_CORES
    )
    # x pre-packed on host: per block, per t a contiguous [128, 2*blk] chunk
    xt_d = nc.dram_tensor(
        "xt", [N_SHARD * 256 * T_MM], mybir.dt.float8e4, kind="ExternalInput"
    ).ap()
    wt_d = nc.dram_tensor(
        "wt", [P, T_MM, J, OUT_F], mybir.dt.float8e4, kind="ExternalInput"
    ).ap()
    out_d = nc.dram_tensor(
        "out", [N_SHARD, OUT_F], mybir.dt.bfloat16, kind="ExternalOutput"
    ).ap()

    with TileContext(nc) as tc:
        with (
            tc.tile_pool(name="const", bufs=1) as cpool,
            tc.tile_pool(name="xin", bufs=4) as xpool,
            tc.tile_pool(name="outp", bufs=4) as opool,
            tc.tile_pool(name="psum", bufs=7, space="PSUM") as ppool,
            tc.tile_pool(name="warm", bufs=1, space="PSUM") as wpool,
        ):
            # dependency-free dummy matmuls on a zeroed SBUF tile: they
            # schedule at engine boot and hold the PE busy so the HAM
            # clock-gate ramp starts before the first real matmul
            scratch = cpool.tile([P, P], mybir.dt.bfloat16)
            nc.gpsimd.memset(scratch[:], 0.0)
            wps = wpool.tile([P, 64], mybir.dt.float32)
            for _ in range(40):
                nc.tensor.matmul(
                    wps[:], lhsT=scratch[:], rhs=scratch[:, :64],
                    start=True, stop=True,
                )

            # constants on the ACT (write) ring so the first x-block
            # read isn't queued behind them on the SP ring
            wt_sb = cpool.tile([P, T_MM, J, OUT_F], mybir.dt.float8e4)
            nc.scalar.dma_start(wt_sb[:], wt_d[:])

            off = 0
            for bi, blk in enumerate(BLOCKS):
                n_sub = blk // P
                x_sb = [
                    xpool.tile([P, J, n_sub, P], mybir.dt.float8e4,
                               tag=f"x{t}", name=f"x{t}")
                    for t in range(T_MM)
                ]
                base = off * 256 * T_MM
                t_sz = blk * 256  # bytes per t chunk
                for t in range(T_MM):
                    src = xt_d[
                        base + t * t_sz:base + (t + 1) * t_sz
                    ].rearrange("(ki f) -> ki f", ki=P)
                    nc.sync.dma_start(
                        x_sb[t][:].rearrange("p j s q -> p (j s q)"), src
                    )
                o_sb = opool.tile([P, n_sub, OUT_F], mybir.dt.bfloat16)
                # rows [off, off+blk) as [p, s, o]: row = off + p*n_sub + s
                # -> contiguous (s, o) run per partition
                dst = out_d[off:off + blk, :].rearrange(
                    "(p s) o -> p s o", s=n_sub
                )
                # write each block in halves so the first half's out-DMA
                # overlaps the second half's matmuls
                h = max(1, min(4, n_sub // 2))
                for half in range((n_sub + h - 1) // h):
                    s0, s1 = half * h, min((half + 1) * h, n_sub)
                    for ns in range(s0, s1):
                        ps = ppool.tile([P, OUT_F], mybir.dt.float32)
                        for t in range(T_MM):
                            # column p covers row off + p*n_sub + ns
                            nc.tensor.matmul(
                                ps[:],
                                lhsT=x_sb[t][:, :, ns, :],
                                rhs=wt_sb[:, t, :, :],
                                start=(t == 0),
                                stop=(t == T_MM - 1),
                                perf_mode=mybir.MatmulPerfMode.DoubleRow,
                            )
                        nc.vector.tensor_copy(o_sb[:, ns, :], ps[:])
                    nc.scalar.dma_start(
                        dst[:, s0:s1, :], o_sb[:, s0:s1, :]
                    )
                off += blk

    nc.finalize()
    return nc


_E4 = ml_dtypes.float8_e4m3


def _q_parts(v):
    """e4m3 RNE quantize (fp32 in/out) + ulp of each element."""
    a = np.abs(v)
    _, e = np.frexp(a)
    qe = np.maximum(e - 4, -9)
    u = np.ldexp(np.ones_like(v, dtype=np.float32), qe)
    q = np.copysign(np.ldexp(np.round(np.ldexp(a, -qe)), qe), v)
    return q.astype(np.float32), u.astype(np.float32)


def _q_fast(v):
    a = np.abs(v)
    _, e = np.frexp(a)
    qe = np.maximum(e - 4, -9)
    return np.copysign(np.ldexp(np.round(np.ldexp(a, -qe)), qe),
                       v).astype(np.float32)


def _encode_rows(xr):
    """[n, 512] fp32 -> (codes [n, 512] fp32 e4m3-exact, d [n, 256]).

    Joint pair encoding: effective x~_2m = a_2m + H*d_m,
    x~_2m+1 = a_2m+1 + KH*d_m. Searches the e4m3-exact dither values
    that align either member's residual to its quantization grid.
    """
    xp = np.ascontiguousarray(xr[:, 0::2])
    xq = np.ascontiguousarray(xr[:, 1::2])
    qp, up = _q_parts(xp)
    qq, uq = _q_parts(xq)
    rp = xp - qp
    rq = xq - qq
    best = (rp * rp + rq * rq).astype(np.float32)  # d = 0 baseline
    bestd = np.zeros(xp.shape, np.float32)
    for k in (-2, -1, 0, 1, 2):
        for draw, scale in (((rp + k * up), H), ((rq + k * uq), KH)):
            d = _q_fast(np.clip(draw / scale, -32.0, 32.0))
            ap = _q_fast(xp - H * d)
            aq = _q_fast(xq - KH * d)
            ep = ap + H * d - xp
            eq = aq + KH * d - xq
            err = ep * ep + eq * eq
            m = err < best
            np.copyto(best, err, where=m)
            np.copyto(bestd, d, where=m)
    ap = _q_fast(xp - H * bestd)
    aq = _q_fast(xq - KH * bestd)
    # pair 255 carries the bias row instead of a dither: plain e4m3
    ap[:, 255] = _q_fast(xr[:, 510])
    aq[:, 255] = _q_fast(xr[:, 511])
    bestd[:, 255] = 1.0
    codes = np.empty_like(xr)
    codes[:, 0::2] = ap
    codes[:, 1::2] = aq
    return codes, bestd


def _pack_x_shard(shard_f32):
    """[N_SHARD, 512] fp32 -> flat fp8 per-block [t][ki, j, ns, p] pack."""
    chunks = []
    off = 0
    for blk in BLOCKS:
        n_sub = blk // P
        codes, d = _encode_rows(shard_f32[off:off + blk, :])
        # codes [p*n_sub + s, f] -> [t, ki, j, s, p]
        c = codes.reshape(P, n_sub, 2, 128, 2).transpose(2, 3, 4, 1, 0)
        dd = d.reshape(P, n_sub, 128, 2).transpose(2, 3, 1, 0)
        blk_flat = np.concatenate(
            [np.ascontiguousarray(c).reshape(-1),
             np.ascontiguousarray(dd).reshape(-1)]
        )
        chunks.append(blk_flat.astype(_E4))
        off += blk
    return np.concatenate(chunks)


def _pack_w(weight, bias):
    wb = np.sign(weight.astype(np.float32)).T       # [i, o]
    wt = np.empty((P, T_MM, J, OUT_F), np.float32)
    for t in range(2):
        wt[:, t, :, :] = wb[256 * t:256 * (t + 1), :].reshape(P, J, OUT_F)
    # dither rhs: slot m=2ki+j -> h*(w_2m + 0.5*w_2m+1); slot 255 = bias
    wpair = wb.reshape(N_PAIR, 2, OUT_F)
    dith = H * wpair[:, 0, :] + KH * wpair[:, 1, :]
    dith[255, :] = np.asarray(bias, np.float32).astype(_E4).astype(np.float32)
    wt[:, 2, :, :] = dith.reshape(P, J, OUT_F)
    return np.ascontiguousarray(wt).astype(_E4)


def kernel(x: np.ndarray, weight: np.ndarray, bias: np.ndarray, **run_kwargs):
    global _nc_cache
    if _nc_cache is None:
        _nc_cache = _build_nc()
    nc = _nc_cache

    x = np.asarray(x)
    wt = _pack_w(np.asarray(weight), np.asarray(bias))

    in_maps = []
    for c in range(N_CORES):
        shard = np.ascontiguousarray(
            x[c * N_SHARD:(c + 1) * N_SHARD, :], dtype=np.float32
        )
        in_maps.append({"xt": _pack_x_shard(shard), "wt": wt})

    res = bass_utils.run_bass_kernel_spmd(
        nc, in_maps, core_ids=list(range(N_CORES)), **run_kwargs
    )
    out = np.empty((N_TOTAL, OUT_F), dtype=np.float32)
    for c in range(N_CORES):
        out[c * N_SHARD:(c + 1) * N_SHARD, :] = res.results[c]["out"].astype(
            np.float32
        )
    if run_kwargs:
        kernel.last_result = res
    return out
